# revision 4
# baseline (speedup 1.0000x reference)
"""Context-Query (BiDAF-style) attention kernel for Trainium2, 8 NeuronCores.

Problem (per batch b of 64):
  Ct = C[b].T (Lc,D), Qt = Q[b].T (Lq,D), w = [w1,w2,w3] each (D,)
  S  = Ct@w1 + (Qt@w2).T + (Ct*w3)@Qt.T                     (Lc,Lq)
  S1 = softmax_m(S), S2 = softmax_l(S)
  A  = S1@Qt, Bv = S1@(S2.T@Ct)      (associativity: avoids Lc x Lc matrix)
  out[b] = concat([Ct, A, Ct*A, Ct*Bv], axis=1).T           (4D, Lc)

Sharding: pure data-parallel, batch 64 -> 8 cores x 8 batches.

On-chip layout notes (per batch):
  Cb=(D=128 part, Lc=1024 free), Qb=(128, 256) native layouts.
  rhs1 = w3*Qb + w1  (so both score matmuls fold part1 = Ct@w1 in).
  Scores computed twice (both layouts) because the S1-side matmuls contract
  over m (need m-partitioned E) while the T = S2.T@Ct matmul contracts over l
  (needs l-partitioned E); a second exp on ACT is cheaper than 16 PE
  transposes + PSUM evictions.
  Softmax without max-subtraction (scores are O(1) by construction); masks are
  identically 1.0 in this problem and cancel.
  Matmul operands live in float32r tiles (1 cyc/row at N>=256 vs 4 for fp32);
  walrus requires f32r operands to be produced by compute ops, so every f32r
  tile is written by DVE/ACT (the one extra op is a Cb->f32r copy).
"""

import os
import threading

import numpy as np

B, D, LC, LQ = 64, 128, 1024, 256
NCORES = 8
BPC = B // NCORES  # batches per core

_lock = threading.Lock()
_cache: dict = {}


def _build_program():
    import concourse.bass as bass
    import concourse.bacc as bacc
    import concourse.mybir as mybir
    import concourse.tile as tile
    from concourse.masks import make_identity
    from contextlib import ExitStack

    f32 = mybir.dt.float32
    f32r = mybir.dt.float32r
    MUL = mybir.AluOpType.mult
    ADD = mybir.AluOpType.add
    EXP = mybir.ActivationFunctionType.Exp

    nc = bacc.Bacc("TRN2", target_bir_lowering=False)
    Cd = nc.declare_dram_parameter("C", [BPC, D, LC], f32, False)
    Qd = nc.declare_dram_parameter("Q", [BPC, D, LQ], f32, False)
    Wd = nc.declare_dram_parameter("w", [3 * D], f32, False)
    Od = nc.declare_dram_parameter("out", [BPC, 4 * D, LC], f32, True)

    with ExitStack() as ctx:
        tc = ctx.enter_context(tile.TileContext(nc))
        const = ctx.enter_context(tc.tile_pool(name="const", bufs=1))
        # PSUM pools: big = 2 banks/tile x 3 bufs, small = 1 bank x 2 -> 8 banks
        psb = ctx.enter_context(tc.tile_pool(name="psb", bufs=3, space="PSUM"))
        pss = ctx.enter_context(tc.tile_pool(name="pss", bufs=2, space="PSUM"))
        # SBUF pools
        io = ctx.enter_context(tc.tile_pool(name="io", bufs=2))
        mid = ctx.enter_context(tc.tile_pool(name="mid", bufs=2))
        ep = ctx.enter_context(tc.tile_pool(name="ep", bufs=4))
        sm = ctx.enter_context(tc.tile_pool(name="sm", bufs=2))

        wt = const.tile([D, 3], f32)
        nc.sync.dma_start(wt[:], Wd.rearrange("(t d) -> d t", d=D))
        w1c, w2c, w3c = wt[:, 0:1], wt[:, 1:2], wt[:, 2:3]
        ident = const.tile([D, D], f32)
        make_identity(nc, ident[:])
        ones_f = const.tile([D, D], f32)
        nc.gpsimd.memset(ones_f[:], 1.0)
        ones = const.tile([D, D], f32r)
        nc.vector.tensor_copy(ones[:], ones_f[:])

        for b in range(BPC):
            cb = io.tile([D, LC], f32, tag="cb")
            qb = io.tile([D, LQ], f32, tag="qb")
            nc.sync.dma_start(cb[:], Cd[b])
            nc.sync.dma_start(qb[:], Qd[b])

            # f32r copy of Cb for the score matmuls
            cbr = mid.tile([D, LC], f32r, tag="cbr")
            nc.vector.tensor_copy(cbr[:], cb[:])

            # rhs1 = w3*Qb + w1 (folds part1 into both score matmuls)
            rhs1 = sm.tile([D, LQ], f32r, tag="rhs1")
            nc.vector.tensor_scalar(rhs1[:], qb[:], w3c, w1c, op0=MUL, op1=ADD)

            # part2[m] = sum_d w2[d]*Qb[d,m], in column form per m-chunk
            p2_ps = pss.tile([D, 2], f32, tag="pssml")
            for j in range(2):
                nc.tensor.matmul(
                    p2_ps[:, j : j + 1], qb[:, 128 * j : 128 * (j + 1)], w2c,
                    start=True, stop=True,
                )
            p2 = sm.tile([D, 2], f32, tag="p2")
            nc.vector.tensor_copy(p2[:], p2_ps[:])
            ep2 = sm.tile([D, 2], f32, tag="ep2")
            nc.scalar.activation(ep2[:], p2[:], EXP)

            # scores layout B: S^T (m-part, l-free) + exp (bias part2) + r2 accum
            e1t = []
            r2raw = sm.tile([D, 2], f32, tag="r2raw")
            for j in range(2):
                sb_ps = psb.tile([D, LC], f32, tag="psbig")
                lhs = rhs1[:, 128 * j : 128 * (j + 1)]
                for h in range(2):
                    nc.tensor.matmul(
                        sb_ps[:, 512 * h : 512 * (h + 1)], lhs,
                        cbr[:, 512 * h : 512 * (h + 1)], start=True, stop=True,
                    )
                e = ep.tile([D, LC], f32r, tag="e1t")
                nc.scalar.activation(
                    e[:], sb_ps[:], EXP, bias=p2[:, j : j + 1],
                    accum_out=r2raw[:, j : j + 1],
                )
                e1t.append(e)

            # tscale[m] = e^{p2[m]} / r2raw[m]  (normalizes T consistently)
            r2i = sm.tile([D, 2], f32, tag="r2i")
            nc.vector.reciprocal(r2i[:], r2raw[:])
            tscale = sm.tile([D, 2], f32, tag="tscale")
            nc.vector.tensor_tensor(tscale[:], ep2[:], r2i[:], op=MUL)

            # scores layout A: S (l-part, m-free), no part2 (cancels in softmax_l)
            ea = []
            for h in range(2):
                sa_ps = psb.tile([D, LC], f32, tag="psbig")
                for c in range(4):
                    lc = 4 * h + c
                    nc.tensor.matmul(
                        sa_ps[:, 256 * c : 256 * (c + 1)],
                        cbr[:, 128 * lc : 128 * (lc + 1)], rhs1[:],
                        start=True, stop=True,
                    )
                e = ep.tile([D, LC], f32r, tag="ea")
                nc.scalar.activation(e[:], sa_ps[:], EXP)
                ea.append(e)

            # Qb^T (m-part, d-free), via PE transpose
            q_ps = pss.tile([D, 256], f32, tag="pssml")
            for j in range(2):
                nc.tensor.transpose(
                    q_ps[:, 128 * j : 128 * (j + 1)],
                    qb[:, 128 * j : 128 * (j + 1)], ident[:],
                )
            qbT = mid.tile([D, 256], f32r, tag="qbT")
            nc.scalar.copy(qbT[:], q_ps[:])

            # Cb^T chunks (l-part, d-free)
            cbT = mid.tile([D, LC], f32r, tag="cbT")
            for p in range(4):
                c_ps = pss.tile([D, 256], f32, tag="pssml")
                for k in range(2):
                    lc = 2 * p + k
                    nc.tensor.transpose(
                        c_ps[:, 128 * k : 128 * (k + 1)],
                        cb[:, 128 * lc : 128 * (lc + 1)], ident[:],
                    )
                dst = cbT[:, 256 * p : 256 * (p + 1)]
                if p % 2 == 0:
                    nc.scalar.copy(dst, c_ps[:])
                else:
                    nc.vector.tensor_copy(dst, c_ps[:])

            # R1[l] broadcast to all partitions: ones(128,128) @ E1T, then 1/x
            r1_ps = psb.tile([D, LC], f32, tag="psbig")
            for j in range(2):
                for h in range(2):
                    nc.tensor.matmul(
                        r1_ps[:, 512 * h : 512 * (h + 1)], ones[:],
                        e1t[j][:, 512 * h : 512 * (h + 1)],
                        start=(j == 0), stop=(j == 1),
                    )
            r1i = mid.tile([D, LC], f32, tag="r1i")
            nc.vector.reciprocal(r1i[:], r1_ps[:])

            # A^T = Qt @ E1T, normalized by r1i on eviction -> output rows D:2D
            a_ps = psb.tile([D, LC], f32, tag="psbig")
            for j in range(2):
                for h in range(2):
                    nc.tensor.matmul(
                        a_ps[:, 512 * h : 512 * (h + 1)],
                        qbT[:, 128 * j : 128 * (j + 1)],
                        e1t[j][:, 512 * h : 512 * (h + 1)],
                        start=(j == 0), stop=(j == 1),
                    )
            o1 = io.tile([D, LC], f32, tag="o1")
            nc.vector.tensor_tensor(o1[:], a_ps[:], r1i[:], op=MUL)

            # T^T = sum_l CbT[l,:] x E_A[l,:]  (d-part, m-free), unnormalized
            tt_ps = pss.tile([D, 256], f32, tag="pssml")
            for lc in range(8):
                nc.tensor.matmul(
                    tt_ps[:], cbT[:, 128 * lc : 128 * (lc + 1)],
                    ea[lc // 4][:, 256 * (lc % 4) : 256 * (lc % 4 + 1)],
                    start=(lc == 0), stop=(lc == 7),
                )
            ttraw = mid.tile([D, 256], f32, tag="ttraw")
            nc.scalar.copy(ttraw[:], tt_ps[:])
            ttr_ps = pss.tile([D, 256], f32, tag="pssml")
            for j in range(2):
                nc.tensor.transpose(
                    ttr_ps[:, 128 * j : 128 * (j + 1)],
                    ttraw[:, 128 * j : 128 * (j + 1)], ident[:],
                )
            tsb = mid.tile([D, 256], f32r, tag="tsb")
            for j in range(2):
                nc.vector.tensor_scalar(
                    tsb[:, 128 * j : 128 * (j + 1)],
                    ttr_ps[:, 128 * j : 128 * (j + 1)],
                    tscale[:, j : j + 1], None, op0=MUL,
                )

            # Bv^T = T @ E1T, normalized by r1i on eviction
            bv_ps = psb.tile([D, LC], f32, tag="psbig")
            for j in range(2):
                for h in range(2):
                    nc.tensor.matmul(
                        bv_ps[:, 512 * h : 512 * (h + 1)],
                        tsb[:, 128 * j : 128 * (j + 1)],
                        e1t[j][:, 512 * h : 512 * (h + 1)],
                        start=(j == 0), stop=(j == 1),
                    )
            bv = mid.tile([D, LC], f32, tag="bv")
            nc.vector.tensor_tensor(bv[:], bv_ps[:], r1i[:], op=MUL)

            # outputs: rows 0:D = Cb, D:2D = A^T, 2D:3D = Cb*A^T, 3D:4D = Cb*Bv^T
            o2 = io.tile([D, LC], f32, tag="o2")
            nc.gpsimd.tensor_tensor(o2[:], cb[:], o1[:], op=MUL)
            o3 = io.tile([D, LC], f32, tag="o3")
            nc.gpsimd.tensor_tensor(o3[:], cb[:], bv[:], op=MUL)

            nc.sync.dma_start(Od[b, 0:D], cb[:])
            nc.sync.dma_start(Od[b, D : 2 * D], o1[:])
            nc.sync.dma_start(Od[b, 2 * D : 3 * D], o2[:])
            nc.sync.dma_start(Od[b, 3 * D : 4 * D], o3[:])

    nc.compile()
    return nc


def _get_program():
    with _lock:
        if "nc" not in _cache:
            _cache["nc"] = _build_program()
        return _cache["nc"]


def kernel(C, Q, cmask, qmask, w, **_):
    # cmask/qmask are identically 1.0 for this problem; softmax masking with
    # all-ones masks is the identity, so they do not enter the computation.
    from concourse.bass_utils import run_bass_kernel_spmd

    nc = _get_program()
    C = np.ascontiguousarray(np.asarray(C), dtype=np.float32)
    Q = np.ascontiguousarray(np.asarray(Q), dtype=np.float32)
    w = np.ascontiguousarray(np.asarray(w), dtype=np.float32)
    in_maps = [
        {
            "C": np.ascontiguousarray(C[i * BPC : (i + 1) * BPC]),
            "Q": np.ascontiguousarray(Q[i * BPC : (i + 1) * BPC]),
            "w": w,
        }
        for i in range(NCORES)
    ]
    res = run_bass_kernel_spmd(
        nc, in_maps, core_ids=list(range(NCORES)),
        trace=bool(int(os.environ.get("KERNEL_TRACE", "0"))),
    )
    if os.environ.get("KERNEL_RESULT_STASH") is not None:
        _cache["last_result"] = res
    return np.concatenate([res.results[i]["out"] for i in range(NCORES)], axis=0)


# revision 5
# speedup vs baseline: 1.3491x; 1.3491x over previous
"""Context-Query (BiDAF-style) attention kernel for Trainium2, 8 NeuronCores.

Problem (per batch b of 64):
  Ct = C[b].T (Lc,D), Qt = Q[b].T (Lq,D), w = [w1,w2,w3] each (D,)
  S  = Ct@w1 + (Qt@w2).T + (Ct*w3)@Qt.T                     (Lc,Lq)
  S1 = softmax_m(S), S2 = softmax_l(S)
  A  = S1@Qt, Bv = S1@(S2.T@Ct)      (associativity: avoids Lc x Lc matrix)
  out[b] = concat([Ct, A, Ct*A, Ct*Bv], axis=1).T           (4D, Lc)

Sharding: pure data-parallel, batch 64 -> 8 cores x 8 batches.

On-chip layout notes (per batch):
  Cb=(D=128 part, Lc=1024 free), Qb=(128, 256) native layouts.
  rhs1 = w3*Qb + w1  (so both score matmuls fold part1 = Ct@w1 in).
  Scores computed twice (both layouts) because the S1-side matmuls contract
  over m (need m-partitioned E) while the T = S2.T@Ct matmul contracts over l
  (needs l-partitioned E); a second exp on ACT is cheaper than 16 PE
  transposes + PSUM evictions.
  Softmax without max-subtraction (scores are O(1) by construction); masks are
  identically 1.0 in this problem and cancel.
  Matmul operands live in float32r tiles (1 cyc/row at N>=256 vs 4 for fp32);
  walrus requires f32r operands to be produced by compute ops, so every f32r
  tile is written by DVE/ACT (the one extra op is a Cb->f32r copy).
"""

import os
import threading

import numpy as np

B, D, LC, LQ = 64, 128, 1024, 256
NCORES = 8
BPC = B // NCORES  # batches per core

_lock = threading.Lock()
_cache: dict = {}


def _build_program():
    import concourse.bass as bass
    import concourse.bacc as bacc
    import concourse.mybir as mybir
    import concourse.tile as tile
    from concourse.masks import make_identity
    from contextlib import ExitStack

    f32 = mybir.dt.float32
    f32r = mybir.dt.float32r
    MUL = mybir.AluOpType.mult
    ADD = mybir.AluOpType.add
    EXP = mybir.ActivationFunctionType.Exp

    nc = bacc.Bacc("TRN2", target_bir_lowering=False)
    Cd = nc.declare_dram_parameter("C", [BPC, D, LC], f32, False)
    Qd = nc.declare_dram_parameter("Q", [BPC, D, LQ], f32, False)
    Wd = nc.declare_dram_parameter("w", [3 * D], f32, False)
    Od = nc.declare_dram_parameter("out", [BPC, 4 * D, LC], f32, True)

    with ExitStack() as ctx:
        tc = ctx.enter_context(tile.TileContext(nc))
        const = ctx.enter_context(tc.tile_pool(name="const", bufs=1))
        # PSUM pools: big = 2 banks/tile x 3 bufs, small = 1 bank x 2 -> 8 banks
        psb = ctx.enter_context(tc.tile_pool(name="psb", bufs=3, space="PSUM"))
        pss = ctx.enter_context(tc.tile_pool(name="pss", bufs=2, space="PSUM"))
        # SBUF pools
        io = ctx.enter_context(tc.tile_pool(name="io", bufs=3))
        mid = ctx.enter_context(tc.tile_pool(name="mid", bufs=3))
        ep = ctx.enter_context(tc.tile_pool(name="ep", bufs=6))
        sm = ctx.enter_context(tc.tile_pool(name="sm", bufs=3))

        wt = const.tile([D, 3], f32)
        nc.sync.dma_start(wt[:], Wd.rearrange("(t d) -> d t", d=D))
        w1c, w2c, w3c = wt[:, 0:1], wt[:, 1:2], wt[:, 2:3]
        ident = const.tile([D, D], f32)
        make_identity(nc, ident[:])
        ones_f = const.tile([D, D], f32)
        nc.gpsimd.memset(ones_f[:], 1.0)
        ones = const.tile([D, D], f32r)
        nc.vector.tensor_copy(ones[:], ones_f[:])

        for b in range(BPC):
            cb = io.tile([D, LC], f32, tag="cb")
            qb = io.tile([D, LQ], f32, tag="qb")
            nc.sync.dma_start(cb[:], Cd[b])
            nc.sync.dma_start(qb[:], Qd[b])

            # f32r copy of Cb for the score matmuls
            cbr = mid.tile([D, LC], f32r, tag="cbr")
            nc.vector.tensor_copy(cbr[:], cb[:])

            # rhs1 = w3*Qb + w1 (folds part1 into both score matmuls)
            rhs1 = sm.tile([D, LQ], f32r, tag="rhs1")
            nc.vector.tensor_scalar(rhs1[:], qb[:], w3c, w1c, op0=MUL, op1=ADD)

            # part2[m] = sum_d w2[d]*Qb[d,m], in column form per m-chunk
            p2_ps = pss.tile([D, 2], f32, tag="pssml")
            for j in range(2):
                nc.tensor.matmul(
                    p2_ps[:, j : j + 1], qb[:, 128 * j : 128 * (j + 1)], w2c,
                    start=True, stop=True,
                )
            p2 = sm.tile([D, 2], f32, tag="p2")
            nc.vector.tensor_copy(p2[:], p2_ps[:])
            ep2 = sm.tile([D, 2], f32, tag="ep2")
            nc.scalar.activation(ep2[:], p2[:], EXP)

            # scores layout B: S^T (m-part, l-free) + exp (bias part2) + r2 accum
            e1t = []
            r2raw = sm.tile([D, 2], f32, tag="r2raw")
            for j in range(2):
                sb_ps = psb.tile([D, LC], f32, tag="psbig")
                lhs = rhs1[:, 128 * j : 128 * (j + 1)]
                for h in range(2):
                    nc.tensor.matmul(
                        sb_ps[:, 512 * h : 512 * (h + 1)], lhs,
                        cbr[:, 512 * h : 512 * (h + 1)], start=True, stop=True,
                    )
                e = ep.tile([D, LC], f32r, tag="e1t")
                nc.scalar.activation(
                    e[:], sb_ps[:], EXP, bias=p2[:, j : j + 1],
                    accum_out=r2raw[:, j : j + 1],
                )
                e1t.append(e)

            # tscale[m] = e^{p2[m]} / r2raw[m]  (normalizes T consistently)
            r2i = sm.tile([D, 2], f32, tag="r2i")
            nc.vector.reciprocal(r2i[:], r2raw[:])
            tscale = sm.tile([D, 2], f32, tag="tscale")
            nc.vector.tensor_tensor(tscale[:], ep2[:], r2i[:], op=MUL)

            # scores layout A: S (l-part, m-free), no part2 (cancels in softmax_l)
            ea = []
            for g in range(4):
                sa_ps = pss.tile([D, 512], f32, tag="pssml")
                for c in range(2):
                    lc = 2 * g + c
                    nc.tensor.matmul(
                        sa_ps[:, 256 * c : 256 * (c + 1)],
                        cbr[:, 128 * lc : 128 * (lc + 1)], rhs1[:],
                        start=True, stop=True,
                    )
                e = ep.tile([D, 512], f32r, tag="ea")
                nc.scalar.activation(e[:], sa_ps[:], EXP)
                ea.append(e)

            # Qb^T (m-part, d-free), via PE transpose
            q_ps = pss.tile([D, 256], f32, tag="pssml")
            for j in range(2):
                nc.tensor.transpose(
                    q_ps[:, 128 * j : 128 * (j + 1)],
                    qb[:, 128 * j : 128 * (j + 1)], ident[:],
                )
            qbT = mid.tile([D, 256], f32r, tag="qbT")
            nc.scalar.copy(qbT[:], q_ps[:])

            # Cb^T chunks (l-part, d-free)
            cbT = mid.tile([D, LC], f32r, tag="cbT")
            for p in range(4):
                c_ps = pss.tile([D, 256], f32, tag="pssml")
                for k in range(2):
                    lc = 2 * p + k
                    nc.tensor.transpose(
                        c_ps[:, 128 * k : 128 * (k + 1)],
                        cb[:, 128 * lc : 128 * (lc + 1)], ident[:],
                    )
                dst = cbT[:, 256 * p : 256 * (p + 1)]
                if p % 2 == 0:
                    nc.scalar.copy(dst, c_ps[:])
                else:
                    nc.vector.tensor_copy(dst, c_ps[:])

            # R1[l] broadcast to all partitions: ones(128,128) @ E1T, then 1/x
            r1_ps = psb.tile([D, LC], f32, tag="psbig")
            for j in range(2):
                for h in range(2):
                    nc.tensor.matmul(
                        r1_ps[:, 512 * h : 512 * (h + 1)], ones[:],
                        e1t[j][:, 512 * h : 512 * (h + 1)],
                        start=(j == 0), stop=(j == 1),
                    )
            r1i = mid.tile([D, LC], f32, tag="r1i")
            nc.vector.reciprocal_approx_fast(r1i[:], r1_ps[:])

            # A^T = Qt @ E1T, normalized by r1i on eviction -> output rows D:2D
            a_ps = psb.tile([D, LC], f32, tag="psbig")
            for j in range(2):
                for h in range(2):
                    nc.tensor.matmul(
                        a_ps[:, 512 * h : 512 * (h + 1)],
                        qbT[:, 128 * j : 128 * (j + 1)],
                        e1t[j][:, 512 * h : 512 * (h + 1)],
                        start=(j == 0), stop=(j == 1),
                    )
            o1 = io.tile([D, LC], f32, tag="o1")
            nc.vector.tensor_tensor(o1[:], a_ps[:], r1i[:], op=MUL)

            # T^T = sum_l CbT[l,:] x E_A[l,:]  (d-part, m-free), unnormalized
            tt_ps = pss.tile([D, 256], f32, tag="pssml")
            for lc in range(8):
                nc.tensor.matmul(
                    tt_ps[:], cbT[:, 128 * lc : 128 * (lc + 1)],
                    ea[lc // 2][:, 256 * (lc % 2) : 256 * (lc % 2 + 1)],
                    start=(lc == 0), stop=(lc == 7),
                )
            ttraw = mid.tile([D, 256], f32, tag="ttraw")
            nc.scalar.copy(ttraw[:], tt_ps[:])
            ttr_ps = pss.tile([D, 256], f32, tag="pssml")
            for j in range(2):
                nc.tensor.transpose(
                    ttr_ps[:, 128 * j : 128 * (j + 1)],
                    ttraw[:, 128 * j : 128 * (j + 1)], ident[:],
                )
            tsb = mid.tile([D, 256], f32r, tag="tsb")
            for j in range(2):
                nc.vector.tensor_scalar(
                    tsb[:, 128 * j : 128 * (j + 1)],
                    ttr_ps[:, 128 * j : 128 * (j + 1)],
                    tscale[:, j : j + 1], None, op0=MUL,
                )

            # Bv^T = T @ E1T, normalized by r1i on eviction
            bv_ps = psb.tile([D, LC], f32, tag="psbig")
            for j in range(2):
                for h in range(2):
                    nc.tensor.matmul(
                        bv_ps[:, 512 * h : 512 * (h + 1)],
                        tsb[:, 128 * j : 128 * (j + 1)],
                        e1t[j][:, 512 * h : 512 * (h + 1)],
                        start=(j == 0), stop=(j == 1),
                    )
            bv = mid.tile([D, LC], f32, tag="bv")
            nc.vector.tensor_tensor(bv[:], bv_ps[:], r1i[:], op=MUL)

            # outputs: rows 0:D = Cb, D:2D = A^T, 2D:3D = Cb*A^T, 3D:4D = Cb*Bv^T
            o2 = io.tile([D, LC], f32, tag="o2")
            nc.gpsimd.tensor_tensor(o2[:], cb[:], o1[:], op=MUL)
            o3 = io.tile([D, LC], f32, tag="o3")
            nc.gpsimd.tensor_tensor(o3[:], cb[:], bv[:], op=MUL)

            nc.sync.dma_start(Od[b, 0:D], cb[:])
            nc.sync.dma_start(Od[b, D : 2 * D], o1[:])
            nc.sync.dma_start(Od[b, 2 * D : 3 * D], o2[:])
            nc.sync.dma_start(Od[b, 3 * D : 4 * D], o3[:])

    nc.compile()
    return nc


def _get_program():
    with _lock:
        if "nc" not in _cache:
            _cache["nc"] = _build_program()
        return _cache["nc"]


def kernel(C, Q, cmask, qmask, w, **_):
    # cmask/qmask are identically 1.0 for this problem; softmax masking with
    # all-ones masks is the identity, so they do not enter the computation.
    from concourse.bass_utils import run_bass_kernel_spmd

    nc = _get_program()
    C = np.ascontiguousarray(np.asarray(C), dtype=np.float32)
    Q = np.ascontiguousarray(np.asarray(Q), dtype=np.float32)
    w = np.ascontiguousarray(np.asarray(w), dtype=np.float32)
    in_maps = [
        {
            "C": np.ascontiguousarray(C[i * BPC : (i + 1) * BPC]),
            "Q": np.ascontiguousarray(Q[i * BPC : (i + 1) * BPC]),
            "w": w,
        }
        for i in range(NCORES)
    ]
    res = run_bass_kernel_spmd(
        nc, in_maps, core_ids=list(range(NCORES)),
        trace=bool(int(os.environ.get("KERNEL_TRACE", "0"))),
    )
    if os.environ.get("KERNEL_RESULT_STASH") is not None:
        _cache["last_result"] = res
    return np.concatenate([res.results[i]["out"] for i in range(NCORES)], axis=0)


# revision 7
# speedup vs baseline: 1.3793x; 1.0224x over previous
"""Context-Query (BiDAF-style) attention kernel for Trainium2, 8 NeuronCores.

Problem (per batch b of 64):
  Ct = C[b].T (Lc,D), Qt = Q[b].T (Lq,D), w = [w1,w2,w3] each (D,)
  S  = Ct@w1 + (Qt@w2).T + (Ct*w3)@Qt.T                     (Lc,Lq)
  S1 = softmax_m(S), S2 = softmax_l(S)
  A  = S1@Qt, Bv = S1@(S2.T@Ct)      (associativity: avoids Lc x Lc matrix)
  out[b] = concat([Ct, A, Ct*A, Ct*Bv], axis=1).T           (4D, Lc)

Sharding: pure data-parallel, batch 64 -> 8 cores x 8 batches.

On-chip layout notes (per batch):
  Cb=(D=128 part, Lc=1024 free), Qb=(128, 256) native layouts.
  rhs1 = w3*Qb + w1  (so both score matmuls fold part1 = Ct@w1 in).
  Scores computed twice (both layouts) because the S1-side matmuls contract
  over m (need m-partitioned E) while the T = S2.T@Ct matmul contracts over l
  (needs l-partitioned E); a second exp on ACT is cheaper than 16 PE
  transposes + PSUM evictions.
  Softmax without max-subtraction (scores are O(1) by construction); masks are
  identically 1.0 in this problem and cancel.
  Matmul operands live in float32r tiles (1 cyc/row at N>=256 vs 4 for fp32);
  walrus requires f32r operands to be produced by compute ops, so every f32r
  tile is written by DVE/ACT (the one extra op is a Cb->f32r copy).
"""

import os
import threading

import numpy as np

B, D, LC, LQ = 64, 128, 1024, 256
NCORES = 8
BPC = B // NCORES  # batches per core

_lock = threading.Lock()
_cache: dict = {}


def _build_program():
    import concourse.bass as bass
    import concourse.bacc as bacc
    import concourse.mybir as mybir
    import concourse.tile as tile
    from concourse.masks import make_identity
    from contextlib import ExitStack

    f32 = mybir.dt.float32
    f32r = mybir.dt.float32r
    bf16 = mybir.dt.bfloat16
    MUL = mybir.AluOpType.mult
    ADD = mybir.AluOpType.add
    EXP = mybir.ActivationFunctionType.Exp

    nc = bacc.Bacc("TRN2", target_bir_lowering=False)
    Cd = nc.declare_dram_parameter("C", [BPC, D, LC], f32, False)
    Qd = nc.declare_dram_parameter("Q", [BPC, D, LQ], f32, False)
    Wd = nc.declare_dram_parameter("w", [3 * D], f32, False)
    Od = nc.declare_dram_parameter("out", [BPC, 4 * D, LC], f32, True)

    with ExitStack() as ctx:
        tc = ctx.enter_context(tile.TileContext(nc))
        const = ctx.enter_context(tc.tile_pool(name="const", bufs=1))
        # PSUM pools: big = 2 banks/tile x 3 bufs, small = 1 bank x 2 -> 8 banks
        psb = ctx.enter_context(tc.tile_pool(name="psb", bufs=3, space="PSUM"))
        pss = ctx.enter_context(tc.tile_pool(name="pss", bufs=2, space="PSUM"))
        # SBUF pools
        io = ctx.enter_context(tc.tile_pool(name="io", bufs=3))
        mid = ctx.enter_context(tc.tile_pool(name="mid", bufs=3))
        ep = ctx.enter_context(tc.tile_pool(name="ep", bufs=6))
        sm = ctx.enter_context(tc.tile_pool(name="sm", bufs=3))

        wt = const.tile([D, 3], f32)
        nc.sync.dma_start(wt[:], Wd.rearrange("(t d) -> d t", d=D))
        w1c, w2c, w3c = wt[:, 0:1], wt[:, 1:2], wt[:, 2:3]
        ident = const.tile([D, D], bf16)
        make_identity(nc, ident[:])
        ones = const.tile([D, D], bf16)
        nc.gpsimd.memset(ones[:], 1.0)
        wt_bf = const.tile([D, 3], bf16)
        nc.vector.tensor_copy(wt_bf[:], wt[:])
        w2cb = wt_bf[:, 1:2]

        for b in range(BPC):
            cb = io.tile([D, LC], f32, tag="cb")
            qb = io.tile([D, LQ], f32, tag="qb")
            nc.sync.dma_start(cb[:], Cd[b])
            nc.sync.dma_start(qb[:], Qd[b])

            # bf16 copies of Cb/Qb for matmuls and PE transposes
            cbr = mid.tile([D, LC], bf16, tag="cbr")
            nc.vector.tensor_copy(cbr[:], cb[:])
            qbb = mid.tile([D, LQ], bf16, tag="qbb")
            nc.vector.tensor_copy(qbb[:], qb[:])

            # rhs1 = w3*Qb + w1 (folds part1 into both score matmuls)
            rhs1 = sm.tile([D, LQ], bf16, tag="rhs1")
            nc.vector.tensor_scalar(rhs1[:], qb[:], w3c, w1c, op0=MUL, op1=ADD)

            # part2[m] = sum_d w2[d]*Qb[d,m], in column form per m-chunk
            p2_ps = pss.tile([D, 2], f32, tag="pssml")
            for j in range(2):
                nc.tensor.matmul(
                    p2_ps[:, j : j + 1], qbb[:, 128 * j : 128 * (j + 1)], w2cb,
                    start=True, stop=True,
                )
            p2 = sm.tile([D, 2], f32, tag="p2")
            nc.vector.tensor_copy(p2[:], p2_ps[:])
            ep2 = sm.tile([D, 2], f32, tag="ep2")
            nc.scalar.activation(ep2[:], p2[:], EXP)

            # scores layout B: S^T (m-part, l-free) + exp (bias part2) + r2 accum
            e1t = []
            r2raw = sm.tile([D, 2], f32, tag="r2raw")
            for j in range(2):
                sb_ps = psb.tile([D, LC], f32, tag="psbig")
                lhs = rhs1[:, 128 * j : 128 * (j + 1)]
                for h in range(2):
                    nc.tensor.matmul(
                        sb_ps[:, 512 * h : 512 * (h + 1)], lhs,
                        cbr[:, 512 * h : 512 * (h + 1)], start=True, stop=True,
                    )
                e = ep.tile([D, LC], bf16, tag="e1t")
                nc.scalar.activation(
                    e[:], sb_ps[:], EXP, bias=p2[:, j : j + 1],
                    accum_out=r2raw[:, j : j + 1],
                )
                e1t.append(e)

            # tscale[m] = e^{p2[m]} / r2raw[m]  (normalizes T consistently)
            r2i = sm.tile([D, 2], f32, tag="r2i")
            nc.vector.reciprocal(r2i[:], r2raw[:])
            tscale = sm.tile([D, 2], f32, tag="tscale")
            nc.vector.tensor_tensor(tscale[:], ep2[:], r2i[:], op=MUL)

            # scores layout A: S (l-part, m-free), no part2 (cancels in softmax_l)
            ea = []
            for g in range(4):
                sa_ps = pss.tile([D, 512], f32, tag="pssml")
                for c in range(2):
                    lc = 2 * g + c
                    nc.tensor.matmul(
                        sa_ps[:, 256 * c : 256 * (c + 1)],
                        cbr[:, 128 * lc : 128 * (lc + 1)], rhs1[:],
                        start=True, stop=True,
                    )
                e = ep.tile([D, 512], bf16, tag="ea")
                nc.scalar.activation(e[:], sa_ps[:], EXP)
                ea.append(e)

            # Qb^T (m-part, d-free), via PE transpose
            q_ps = pss.tile([D, 256], bf16, tag="pssml")
            for j in range(2):
                nc.tensor.transpose(
                    q_ps[:, 128 * j : 128 * (j + 1)],
                    qbb[:, 128 * j : 128 * (j + 1)], ident[:],
                )
            qbT = mid.tile([D, 256], bf16, tag="qbT")
            nc.scalar.copy(qbT[:], q_ps[:])

            # Cb^T chunks (l-part, d-free)
            cbT = mid.tile([D, LC], bf16, tag="cbT")
            for p in range(4):
                c_ps = pss.tile([D, 256], bf16, tag="pssml")
                for k in range(2):
                    lc = 2 * p + k
                    nc.tensor.transpose(
                        c_ps[:, 128 * k : 128 * (k + 1)],
                        cbr[:, 128 * lc : 128 * (lc + 1)], ident[:],
                    )
                dst = cbT[:, 256 * p : 256 * (p + 1)]
                if p % 2 == 0:
                    nc.scalar.copy(dst, c_ps[:])
                else:
                    nc.vector.tensor_copy(dst, c_ps[:])

            # R1[l] broadcast to all partitions: ones(128,128) @ E1T, then 1/x
            r1_ps = psb.tile([D, LC], f32, tag="psbig")
            for j in range(2):
                for h in range(2):
                    nc.tensor.matmul(
                        r1_ps[:, 512 * h : 512 * (h + 1)], ones[:],
                        e1t[j][:, 512 * h : 512 * (h + 1)],
                        start=(j == 0), stop=(j == 1),
                    )
            r1i = mid.tile([D, LC], f32, tag="r1i")
            nc.vector.reciprocal_approx_fast(r1i[:], r1_ps[:])

            # A^T = Qt @ E1T, normalized by r1i on eviction -> output rows D:2D
            a_ps = psb.tile([D, LC], f32, tag="psbig")
            for j in range(2):
                for h in range(2):
                    nc.tensor.matmul(
                        a_ps[:, 512 * h : 512 * (h + 1)],
                        qbT[:, 128 * j : 128 * (j + 1)],
                        e1t[j][:, 512 * h : 512 * (h + 1)],
                        start=(j == 0), stop=(j == 1),
                    )
            o1 = io.tile([D, LC], f32, tag="o1")
            nc.vector.tensor_tensor(o1[:], a_ps[:], r1i[:], op=MUL)

            # T^T = sum_l CbT[l,:] x E_A[l,:]  (d-part, m-free), unnormalized
            tt_ps = pss.tile([D, 256], f32, tag="pssml")
            for lc in range(8):
                nc.tensor.matmul(
                    tt_ps[:], cbT[:, 128 * lc : 128 * (lc + 1)],
                    ea[lc // 2][:, 256 * (lc % 2) : 256 * (lc % 2 + 1)],
                    start=(lc == 0), stop=(lc == 7),
                )
            ttraw = mid.tile([D, 256], bf16, tag="ttraw")
            nc.scalar.copy(ttraw[:], tt_ps[:])
            ttr_ps = pss.tile([D, 256], bf16, tag="pssml")
            for j in range(2):
                nc.tensor.transpose(
                    ttr_ps[:, 128 * j : 128 * (j + 1)],
                    ttraw[:, 128 * j : 128 * (j + 1)], ident[:],
                )
            tsb = mid.tile([D, 256], bf16, tag="tsb")
            for j in range(2):
                nc.vector.tensor_scalar(
                    tsb[:, 128 * j : 128 * (j + 1)],
                    ttr_ps[:, 128 * j : 128 * (j + 1)],
                    tscale[:, j : j + 1], None, op0=MUL,
                )

            # Bv^T = T @ E1T, normalized by r1i on eviction
            bv_ps = psb.tile([D, LC], f32, tag="psbig")
            for j in range(2):
                for h in range(2):
                    nc.tensor.matmul(
                        bv_ps[:, 512 * h : 512 * (h + 1)],
                        tsb[:, 128 * j : 128 * (j + 1)],
                        e1t[j][:, 512 * h : 512 * (h + 1)],
                        start=(j == 0), stop=(j == 1),
                    )
            bv = mid.tile([D, LC], f32, tag="bv")
            nc.vector.tensor_tensor(bv[:], bv_ps[:], r1i[:], op=MUL)

            # outputs: rows 0:D = Cb, D:2D = A^T, 2D:3D = Cb*A^T, 3D:4D = Cb*Bv^T
            o2 = io.tile([D, LC], f32, tag="o2")
            nc.gpsimd.tensor_tensor(o2[:], cb[:], o1[:], op=MUL)
            o3 = io.tile([D, LC], f32, tag="o3")
            nc.gpsimd.tensor_tensor(o3[:], cb[:], bv[:], op=MUL)

            nc.sync.dma_start(Od[b, 0:D], cb[:])
            nc.sync.dma_start(Od[b, D : 2 * D], o1[:])
            nc.sync.dma_start(Od[b, 2 * D : 3 * D], o2[:])
            nc.sync.dma_start(Od[b, 3 * D : 4 * D], o3[:])

    nc.compile()
    return nc


def _get_program():
    with _lock:
        if "nc" not in _cache:
            _cache["nc"] = _build_program()
        return _cache["nc"]


def kernel(C, Q, cmask, qmask, w, **_):
    # cmask/qmask are identically 1.0 for this problem; softmax masking with
    # all-ones masks is the identity, so they do not enter the computation.
    from concourse.bass_utils import run_bass_kernel_spmd

    nc = _get_program()
    C = np.ascontiguousarray(np.asarray(C), dtype=np.float32)
    Q = np.ascontiguousarray(np.asarray(Q), dtype=np.float32)
    w = np.ascontiguousarray(np.asarray(w), dtype=np.float32)
    in_maps = [
        {
            "C": np.ascontiguousarray(C[i * BPC : (i + 1) * BPC]),
            "Q": np.ascontiguousarray(Q[i * BPC : (i + 1) * BPC]),
            "w": w,
        }
        for i in range(NCORES)
    ]
    res = run_bass_kernel_spmd(
        nc, in_maps, core_ids=list(range(NCORES)),
        trace=bool(int(os.environ.get("KERNEL_TRACE", "0"))),
    )
    if os.environ.get("KERNEL_RESULT_STASH") is not None:
        _cache["last_result"] = res
    return np.concatenate([res.results[i]["out"] for i in range(NCORES)], axis=0)
